# revision 27
# baseline (speedup 1.0000x reference)
"""MoE router gate (DeepSeek-V3 style) on 8 Trainium2 NeuronCores.

Math (per token):
  logits = x @ w.T            [N=16384, E=256], D=7168, fp32
  scores = sigmoid(logits)
  s      = scores + bias
  group top-2 sums over 8 groups of 32 -> keep top-4 groups
  indices = top-8 of s within kept groups
  weights = renormalize(scores[indices]) * 2.5

Sharding: data-parallel over tokens (2048/core); w+bias replicated.

GEMM strategy (2.0 fp16-pass-equivalents):
  x = xh + 2^-11 * x1        xh fp16, x1 fp8(e4m3)    (host split)
  w = wh + 2^-11 * wl        wh fp16, wl8 fp8(e4m3)
  logits = xh@wh  (fp16 pass, 256-col moving, x-stationary)
         + 2^-11 * (x1@wh + x@wl)   [exact algebra: sum == x@w]
  The correction is ONE fp8 DoubleRow pass with the plane pair
  (x1*a)(wh/a) + (x*c)(wl*2^11/c), w-stationary over 512-token blocks,
  output [exp, tok] transposed back via PE identity matmuls (fp16).
  Scales a=0.5, c=0.25 minimize e4m3 quantization noise (subnormal
  balance); products are scale-free so the merge is logits+psc/2^11.
  Simulated on the real inputs: 11/16384 index flips, rel_err 8.5e-3.
"""

import sys
import threading

sys.path.insert(0, "/opt/trn_rl_repo")

import numpy as np
import ml_dtypes

import concourse.bacc as bacc
import concourse.mybir as mybir
import concourse.tile as tile
from concourse.bass_utils import run_bass_kernel_spmd

N_TOK = 16384
D = 7168
E = 256
N_CORES = 8
NSH = N_TOK // N_CORES          # 2048 tokens per core
TOK_TILE = 128
N_TILES = NSH // TOK_TILE       # 16
KC = 128                        # contraction chunk
N_KC = D // KC                  # 56
BLK = 512                       # corr token block
N_BLK = NSH // BLK              # 4
N_GROUPS = 8
GSIZE = E // N_GROUPS           # 32
TOPK = 8
ROUTE_SCALE = 2.5
SPLIT_SCALE = 2048.0            # 2^11
A_SCALE = 0.5                   # x1-plane fp8 scale split
C_SCALE = 0.25                  # x-plane fp8 scale split
NEG_BIG = 1.0e30
N_WARM = 24                     # PE warmup matmuls (HAM ramp) during DMA wait

# chunk groups for DMA-granular dependencies
GS = [2, 6] + [8] * 6
GOFF = [sum(GS[:i]) for i in range(len(GS))]
NG = len(GS)
C2G = []
for _gi, _n in enumerate(GS):
    C2G += [(_gi, _c) for _c in range(_n)]

# corr schedule: block k's 112 DoubleRow instrs spread over tiles
# 3k..3k+2 in h-major order (all expert-half-0 chunks, then half-1), so
# psc[0] is drained mid-block and neither psc bank is ever a boundary
# stall. The first tile is light so the startup DMA front-load has
# slack. Tiles 12-15 are corr-free, so all routing for the last 4 tiles
# drains inline and the tail is one chain. j' = h*56 + cc.
CORR_JS = [(0, 16), (16, 64), (64, 112)]
GP_OFFLOAD = True               # run part of routing on GpSimd

_cached = {}


def _build_nc():
    fp16 = mybir.dt.float16
    fp8 = mybir.dt.float8e4
    f32 = mybir.dt.float32
    u32 = mybir.dt.uint32

    nc = bacc.Bacc(trn_type="TRN2", target_bir_lowering=False)

    # pretiled dram layouts: partition dim = contraction rows (p = d%128)
    xh_d = nc.dram_tensor("xh", [N_TILES * 128, N_KC * TOK_TILE], fp16,
                          kind="ExternalInput")
    w_d = nc.dram_tensor("w", [128, N_KC * E], fp16, kind="ExternalInput")
    w8_d = nc.dram_tensor("w8", [128, N_KC * 2 * E], fp8, kind="ExternalInput")
    cx_d = nc.dram_tensor("cx", [N_BLK * 128, N_KC * 2 * BLK], fp8,
                          kind="ExternalInput")
    bias_d = nc.dram_tensor("bias", [128, E], f32, kind="ExternalInput")
    ident_d = nc.dram_tensor("ident", [128, 128], fp16, kind="ExternalInput")
    wts_d = nc.dram_tensor("wts", [NSH, TOPK], f32, kind="ExternalOutput")
    idx_d = nc.dram_tensor("idx", [NSH, TOPK], mybir.dt.int32, kind="ExternalOutput")

    with tile.TileContext(nc) as tc:
        with (
            tc.tile_pool(name="wpool", bufs=1) as wpool,
            tc.tile_pool(name="xpool", bufs=2) as xpool,
            tc.tile_pool(name="cxpool", bufs=2) as cxpool,
            tc.tile_pool(name="spool", bufs=2) as spool,
            tc.tile_pool(name="tiny", bufs=2) as tiny,
            tc.tile_pool(name="sct", bufs=2) as sctp,
            tc.tile_pool(name="psum1", bufs=4, space="PSUM") as ps1pool,
            tc.tile_pool(name="psumc", bufs=1, space="PSUM") as pscpool,
            tc.tile_pool(name="psumt", bufs=2, space="PSUM") as pstpool,
        ):
            # --- PE warmup: dummy matmuls with no DMA deps keep the HAM
            # activity window busy so real matmuls start at 2.4 GHz.
            warm_sb = wpool.tile([128, 128], fp16, tag="warm")
            nc.vector.memset(warm_sb[:, :], 0.0)
            ps_warm = ps1pool.tile([128, E], f32, tag="ps1", name="warm")
            for i in range(N_WARM):
                nc.tensor.matmul(
                    ps_warm[:, 0:128], warm_sb[:, :], warm_sb[:, :],
                    start=(i == 0), stop=(i == N_WARM - 1),
                )

            # --- resident weights / bias ---
            bias_sb = wpool.tile([128, E], f32, tag="bias")
            nc.scalar.dma_start(bias_sb[:, :], bias_d[:, :])

            cx_blocks = [None] * N_BLK
            xh_tiles = [None] * N_TILES

            def load_xh_tile(t):
                tiles = []
                for g in range(NG):
                    xhg = xpool.tile(
                        [128, GS[g], TOK_TILE], fp16, tag=f"xh{g}", bufs=2
                    )
                    nc.sync.dma_start(
                        xhg[:, :, :],
                        xh_d[
                            t * 128 : (t + 1) * 128,
                            GOFF[g] * TOK_TILE : (GOFF[g] + GS[g]) * TOK_TILE,
                        ].rearrange("p (c n) -> p c n", c=GS[g]),
                    )
                    tiles.append(xhg)
                xh_tiles[t] = tiles

            def load_cx_block(b, groups=range(NG)):
                if cx_blocks[b] is None:
                    cx_blocks[b] = [None] * NG
                for g in groups:
                    # g6/g7 single-buffered (SBUF pressure): issued only after
                    # the previous block's corr freed them (t=4,8,12).
                    cg = cxpool.tile(
                        [128, GS[g], 2, BLK], fp8, tag=f"cx{g}",
                        bufs=2 if g < 6 else 1,
                    )
                    nc.sync.dma_start(
                        cg[:, :, :, :],
                        cx_d[
                            b * 128 : (b + 1) * 128,
                            GOFF[g] * 2 * BLK : (GOFF[g] + GS[g]) * 2 * BLK,
                        ].rearrange("p (c t n) -> p c t n", c=GS[g], t=2),
                    )
                    cx_blocks[b][g] = cg

            w8_g = [None] * NG

            def load_w8(groups):
                for g in groups:
                    w8g = wpool.tile(
                        [128, GS[g], 2, E], fp8, tag=f"w8{g}", bufs=1, name="w8g"
                    )
                    nc.sync.dma_start(
                        w8g[:, :, :, :],
                        w8_d[
                            :, GOFF[g] * 2 * E : (GOFF[g] + GS[g]) * 2 * E
                        ].rearrange("p (c t e) -> p c t e", c=GS[g], t=2),
                    )
                    w8_g[g] = w8g

            # startup: tile 0 (pure hi) needs only w + xh0; w8/cx0 for the
            # first corr tile (t=1) follow, tails deferred into tile 0
            w_g = []
            xh_tiles[0] = []
            for g in range(NG):
                wg = wpool.tile([128, GS[g], E], fp16, tag=f"w{g}", bufs=1)
                nc.sync.dma_start(
                    wg[:, :, :],
                    w_d[:, GOFF[g] * E : (GOFF[g] + GS[g]) * E].rearrange(
                        "p (c e) -> p c e", c=GS[g]
                    ),
                )
                w_g.append(wg)
                xhg = xpool.tile([128, GS[g], TOK_TILE], fp16, tag=f"xh{g}", bufs=2)
                nc.sync.dma_start(
                    xhg[:, :, :],
                    xh_d[0:128, GOFF[g] * TOK_TILE : (GOFF[g] + GS[g]) * TOK_TILE]
                    .rearrange("p (c n) -> p c n", c=GS[g]),
                )
                xh_tiles[0].append(xhg)
            load_w8(range(3))
            load_cx_block(0, groups=range(3))
            ident_sb = wpool.tile([128, 128], fp16, tag="ident")
            nc.scalar.dma_start(ident_sb[:, :], ident_d[:, :])

            sct_by_block = [None] * N_BLK
            cur_psc = {}
            cur_sct = {}

            def emit_corr_instr(b, j):
                """j-th of 112 DoubleRow corr instrs for block b (h-major)."""
                h, cc = j // N_KC, j % N_KC
                g, ci = C2G[cc]
                if cc == 0:
                    cur_psc[h] = pscpool.tile(
                        [128, BLK], f32, tag=f"psc{h}", bufs=1, name=f"psc{h}"
                    )
                nc.tensor.matmul(
                    cur_psc[h][:, :],
                    w8_g[g][:, ci, :, h * 128 : (h + 1) * 128],
                    cx_blocks[b][g][:, ci, :, :],
                    start=(cc == 0),
                    stop=(cc == N_KC - 1),
                    perf_mode=mybir.MatmulPerfMode.DoubleRow,
                )
                if cc == N_KC - 1:
                    # drain this expert half to fp16 SBUF, freeing the psc
                    # bank mid-block (h=0) / publishing the block (h=1)
                    if h == 0:
                        cur_sct[b] = sctp.tile(
                            [128, 2, BLK], fp16, tag="sct", bufs=2, name="sct"
                        )
                    nc.scalar.activation(
                        cur_sct[b][:, h, :], cur_psc[h][:, :],
                        mybir.ActivationFunctionType.Copy,
                    )
                    if h == 1:
                        sct_by_block[b] = cur_sct.pop(b)

            ps1_by_tile = [None] * N_TILES

            def emit_routing(t):
                ts = t * TOK_TILE
                ps1 = ps1_by_tile[t]
                sct = sct_by_block[t // 4]
                bo = (t % 4) * TOK_TILE
                veng = nc.vector
                geng = nc.gpsimd if GP_OFFLOAD else nc.vector

                pst = pstpool.tile([128, 2, 128], fp16, tag="pst", bufs=2)
                for h in range(2):
                    nc.tensor.matmul(
                        pst[:, h, :],
                        sct[:, h, bo : bo + TOK_TILE],
                        ident_sb[:, :],
                        is_transpose=True,
                        start=(h == 0),
                        stop=(h == 1),
                    )
                cT = spool.tile([128, E], f32, tag="cT")
                nc.scalar.activation(
                    cT[:, :], pst[:, :, :].rearrange("p h n -> p (h n)"),
                    mybir.ActivationFunctionType.Copy,
                )
                logits = spool.tile([128, E], f32, tag="logits")
                veng.scalar_tensor_tensor(
                    logits[:, :], cT[:, :], 1.0 / SPLIT_SCALE, ps1[:, :],
                    op0=mybir.AluOpType.mult, op1=mybir.AluOpType.add,
                )

                scores = spool.tile([128, E], f32, tag="scores")
                nc.scalar.activation(
                    scores[:, :], logits[:, :], mybir.ActivationFunctionType.Sigmoid
                )
                s = spool.tile([128, E], f32, tag="s")
                veng.tensor_add(s[:, :], scores[:, :], bias_sb[:, :])

                # group top-2 sums
                gtop = tiny.tile([128, N_GROUPS, 8], f32, tag="gtop")
                for g in range(N_GROUPS):
                    veng.max(gtop[:, g, :], s[:, g * GSIZE : (g + 1) * GSIZE])
                gs = tiny.tile([128, N_GROUPS], f32, tag="gs")
                veng.tensor_add(gs[:, :], gtop[:, :, 0], gtop[:, :, 1])

                gsort = tiny.tile([128, 8], f32, tag="gsort")
                veng.max(gsort[:, :], gs[:, :])
                # amask = 0 for kept groups (gs >= 4th group score), -BIG else
                amask = tiny.tile([128, N_GROUPS], f32, tag="amask")
                veng.tensor_scalar(
                    amask[:, :], gs[:, :], gsort[:, 3:4], -NEG_BIG,
                    op0=mybir.AluOpType.is_lt, op1=mybir.AluOpType.mult,
                )

                smask = spool.tile([128, N_GROUPS, GSIZE], f32, tag="smask")
                geng.tensor_tensor(
                    smask[:, :, :],
                    s[:, :].rearrange("p (g e) -> p g e", g=N_GROUPS),
                    amask[:, :].unsqueeze(-1).broadcast_to([128, N_GROUPS, GSIZE]),
                    op=mybir.AluOpType.add,
                )

                smask2 = smask[:, :, :].rearrange("p g e -> p (g e)")
                top8v = tiny.tile([128, TOPK], f32, tag="top8v")
                veng.max(top8v[:, :], smask2)
                top8i = tiny.tile([128, TOPK], u32, tag="top8i")
                veng.max_index(top8i[:, :], top8v[:, :], smask2)

                # extract scores at selected positions, aligned to top8v order
                wsel = tiny.tile([128, TOPK], f32, tag="wsel")
                scratch = spool.tile([128, E], f32, tag="scratch", bufs=1)
                scratch2 = spool.tile([128, E], f32, tag="scratch2", bufs=1)
                for j in range(TOPK):
                    eng, scr = (
                        (veng, scratch) if j % 2 == 0 else (veng, scratch2)
                    )
                    eng.scalar_tensor_tensor(
                        scr[:, :], smask2, top8v[:, j : j + 1], scores[:, :],
                        op0=mybir.AluOpType.is_equal, op1=mybir.AluOpType.mult,
                        accum_out=wsel[:, j : j + 1],
                    )

                ssum = tiny.tile([128, 1], f32, tag="ssum")
                veng.reduce_sum(ssum[:, :], wsel[:, :], axis=mybir.AxisListType.X)
                rec = tiny.tile([128, 1], f32, tag="rec")
                veng.reciprocal(rec[:, :], ssum[:, :])
                wout = tiny.tile([128, TOPK], f32, tag="wout")
                veng.tensor_scalar(
                    wout[:, :], wsel[:, :], rec[:, 0:1], ROUTE_SCALE,
                    op0=mybir.AluOpType.mult, op1=mybir.AluOpType.mult,
                )

                nc.sync.dma_start(wts_d[ts : ts + TOK_TILE, :], wout[:, :])
                nc.sync.dma_start(
                    idx_d[ts : ts + TOK_TILE, :],
                    top8i[:, :].bitcast(mybir.dt.int32),
                )

            pending = []
            for t in range(N_TILES):
                if t + 1 < N_TILES:
                    load_xh_tile(t + 1)
                if t == 0:
                    load_w8(range(3, NG))
                    load_cx_block(0, groups=range(3, NG))
                # stagger the next cx block's groups to match consumption
                if t in (2, 5, 8):
                    load_cx_block((t + 1) // 3, groups=range(3))
                if t in (3, 6, 9):
                    load_cx_block(t // 3, groups=range(3, 6))
                if t in (4, 7, 10):
                    load_cx_block((t - 1) // 3, groups=[6, 7])

                if 1 <= t <= 12:
                    # this tile emits corr instrs [j0, j1) of block b at
                    # evenly spaced spots over the 56 hi chunks
                    b = (t - 1) // 3
                    j0, j1 = CORR_JS[(t - 1) % 3]
                    kk = j1 - j0
                    corr_of_chunk = {
                        ((m + 1) * N_KC) // kk - 1: j0 + m for m in range(kk)
                    }
                else:
                    b = None
                    corr_of_chunk = {}

                ps1 = ps1pool.tile([128, E], f32, tag="ps1")
                ps1_by_tile[t] = ps1
                xh_t = xh_tiles[t]
                for c in range(N_KC):
                    g, ci = C2G[c]
                    nc.tensor.matmul(
                        ps1[:, :],
                        xh_t[g][:, ci, :],
                        w_g[g][:, ci, :],
                        start=(c == 0),
                        stop=(c == N_KC - 1),
                    )
                    j = corr_of_chunk.get(c)
                    if j is not None:
                        emit_corr_instr(b, j)

                pending.append(t)
                while pending and sct_by_block[pending[0] // 4] is not None:
                    emit_routing(pending.pop(0))
    nc.finalize()
    return nc


def _host_prep(x, weight, bias):
    """fp16 hi + scaled fp8 correction planes, pretiled per-core shards."""
    x = np.asarray(x, dtype=np.float32)
    weight = np.asarray(weight, dtype=np.float32)
    bias = np.asarray(bias, dtype=np.float32)
    f8 = ml_dtypes.float8_e4m3

    wh = weight.astype(np.float16)
    whf = wh.astype(np.float32)
    # w_d [128, 56*256]: (p, c, e) = wh[e, c*128+p]
    w_tiled = np.ascontiguousarray(
        whf.astype(np.float16).T.reshape(N_KC, 128, E).transpose(1, 0, 2)
        .reshape(128, N_KC * E)
    )
    # fp8 stationary planes: plane0 = e4m3(wh/a), plane1 = e4m3(wl*2^11/c)
    wh8s = (whf / A_SCALE).astype(f8)                       # [E, D]
    wl8s = ((weight - whf) * SPLIT_SCALE / C_SCALE).astype(f8)
    w8_pl = np.stack(
        [
            wh8s.T.reshape(N_KC, 128, E).transpose(1, 0, 2),
            wl8s.T.reshape(N_KC, 128, E).transpose(1, 0, 2),
        ],
        axis=2,
    )  # [128, 56, 2, 256]
    w8_tiled = np.ascontiguousarray(w8_pl.reshape(128, N_KC * 2 * E))

    bias_rep = np.ascontiguousarray(np.broadcast_to(bias[None, :], (128, E)))
    ident = np.eye(128, dtype=np.float16)

    in_maps = [None] * N_CORES

    def prep_core(cid):
        xs = x[cid * NSH : (cid + 1) * NSH, :]              # [2048, 7168]
        xh16 = xs.astype(np.float16)
        xhf = xh16.astype(np.float32)
        # xh_d [16*128, 56*128]: (t, p, c, n) = xh16[t*128+n, c*128+p]
        xh_tiled = np.ascontiguousarray(
            xh16.reshape(N_TILES, TOK_TILE, N_KC, 128)
            .transpose(0, 3, 2, 1)
            .reshape(N_TILES * 128, N_KC * TOK_TILE)
        )
        x1s = ((xs - xhf) * (SPLIT_SCALE * A_SCALE)).astype(f8)
        x8s = (xs * C_SCALE).astype(f8)
        # cx_d [4*128, 56*2*512]: (b, p, c, t, n) = plane_t[b*512+n, c*128+p]
        cx = np.stack(
            [
                x1s.reshape(N_BLK, BLK, N_KC, 128).transpose(0, 3, 2, 1),
                x8s.reshape(N_BLK, BLK, N_KC, 128).transpose(0, 3, 2, 1),
            ],
            axis=3,
        )  # [4, 128, 56, 2, 512]
        cx_tiled = np.ascontiguousarray(cx.reshape(N_BLK * 128, N_KC * 2 * BLK))
        in_maps[cid] = {
            "xh": xh_tiled,
            "w": w_tiled,
            "w8": w8_tiled,
            "cx": cx_tiled,
            "bias": bias_rep,
            "ident": ident,
        }

    threads = [threading.Thread(target=prep_core, args=(c,)) for c in range(N_CORES)]
    for th in threads:
        th.start()
    for th in threads:
        th.join()
    return in_maps


def kernel(x, weight, bias, _trace=False):
    if "nc" not in _cached:
        _cached["nc"] = _build_nc()
    nc = _cached["nc"]
    in_maps = _host_prep(x, weight, bias)
    res = run_bass_kernel_spmd(
        nc, in_maps, core_ids=list(range(N_CORES)), trace=_trace
    )
    _cached["last_result"] = res
    wts = np.concatenate([r["wts"] for r in res.results], axis=0)
    idx = np.concatenate([r["idx"] for r in res.results], axis=0)
    return wts, idx
